# revision 12
# baseline (speedup 1.0000x reference)
"""Bass/Trainium2 kernel for nn_Attn_1185410973711 (additive attention scores).

Computation (reference, fp32):
    W_s = W_attn[:, :H]; W_e = W_attn[:, H:]
    energy  = tanh(output @ W_s.T [:,None,:] + einsum('bse,he->bsh', enc, W_e) + b_attn)
    scores  = einsum('bsh,h->bs', energy, v) - 1000*(mask==0)
    out     = softmax(scores, axis=-1)           # [B, 1, S]

Strategy: data-parallel over batch B=32 across 8 NeuronCores (4 batches per
core); W_attn/b_attn/v replicated. Host-side marshalling pre-transposes
encoder_outputs to [e, s] layout (pure layout change) so the kernel streams
contraction-ready tiles straight from DRAM with zero on-chip transposes.
The dominant matmul (enc_proj, 8.6 GFLOP/core) runs in bf16 (enc and W_e
rounded on the host) with fp32 PSUM accumulation, k-contiguous per PSUM
bank; the small state projection stays float32r. tanh(+state/bias) is
fused on the scalar engine with per-partition bias; the v-dot is a second
M=1 bf16 matmul; softmax runs on partitions {0,32,64,96} at the tail.
"""

import numpy as np

B, S, H = 32, 2048, 512
E2 = 2 * H            # 1024, encoder feature dim
N_CORES = 8
BPC = B // N_CORES    # 4 batches per core
NK = E2 // 128        # 8 contraction tiles
NHT = H // 128        # 4 hidden tiles
SB = 512              # s-block (moving free dim)
NSB = S // SB         # 4 s-blocks
SH = S // 2           # half-batch s columns per enc DMA


def _split_drain_context(nc):
    """TileContext subclass working around two walrus limits in this build:
    the kernel-tail drain and several opcodes (notably the self-loading
    fp32 matmul) reject instructions carrying more than one semaphore
    wait. See enforce_wait_limit()."""
    import concourse.tile as tile
    from concourse.vector_clock import ScopedClock

    class TileContextSplitDrain(tile.TileContext):
        def _drain_and_barrier(self, tick_clock, wait_clock):
            probe = self.nc.sync.nop(nofuse=True, hint="tail_wait_probe")
            wait_clock.add_sem_waits(
                probe.ins, ScopedClock({None: tick_clock.global_clock})
            )
            si = probe.ins.sync_info
            waits = list(si.on_wait or []) if si is not None else []
            if si is not None:
                si.on_wait.clear()
            by_name = {h.name: h for h in self.sems.allocated().values()}
            for w in waits:
                h = by_name.get(w.ant_name)
                assert h is not None, f"missing semaphore handle for {w.ant_name}"
                self.nc.sync.wait_ge(h, w.wait_value)
            self.nc.sync.drain()
            self.nc.all_engine_barrier()
            popped = self.nc._tile_sem_poison_stack.pop()
            assert popped is self._sem_poison
            self.nc.clear_and_free_semaphores(list(self.sems.allocated().values()))
            self.nc.all_engine_barrier()

    return TileContextSplitDrain(nc)


def enforce_wait_limit(nc, limit=1):
    """Hoist excess semaphore waits onto inserted same-engine event-sem wait
    instructions placed immediately before the over-budget instruction.
    In-order engine execution makes an earlier wait strictly conservative,
    so this is always sound."""
    import copy

    template = None
    for fn in nc.m.functions:
        for bb in fn.blocks:
            for ins in bb.instructions:
                if type(ins).__name__ == "InstEventSemaphore":
                    si = ins.sync_info
                    if si and si.on_wait and len(si.on_wait) == 1:
                        template = ins
                        break
            if template:
                break
        if template:
            break

    n_new = 0
    for fn in nc.m.functions:
        for bb in fn.blocks:
            il = bb.instructions
            new_il = []
            changed = False
            for ins in il:
                si = ins.sync_info
                waits = list(si.on_wait) if si and si.on_wait else []
                if len(waits) > limit and type(ins).__name__ != "InstEventSemaphore":
                    assert template is not None, "no event-sem template found"
                    for w in waits[limit:]:
                        c = copy.deepcopy(template)
                        n_new += 1
                        c.name = f"I-waitfix-{n_new}"
                        c.engine = ins.engine
                        csi = c.sync_info
                        csi.on_wait.clear()
                        csi.on_wait.append(w)
                        csi.on_update.clear()
                        new_il.append(c)
                    si.on_wait.clear()
                    for w in waits[:limit]:
                        si.on_wait.append(w)
                    changed = True
                new_il.append(ins)
            if changed:
                il[:] = new_il
    return n_new


def build_nc(reps=1):
    """Build the per-core Bass program. reps>1 wraps the steady-state body in
    a For_i loop re-running the identical computation (for timing)."""
    import concourse.bass as bass
    from concourse import mybir

    f32 = mybir.dt.float32
    f32r = mybir.dt.float32r
    bf16 = mybir.dt.bfloat16
    Tanh = mybir.ActivationFunctionType.Tanh
    Exp = mybir.ActivationFunctionType.Exp

    nc = bass.Bass("TRN2", target_bir_lowering=False, debug=False)

    encT_d = nc.dram_tensor("encT", [BPC, NK, 128, S], bf16, kind="ExternalInput")
    weT_d = nc.dram_tensor("weT", [2 * H, H], bf16, kind="ExternalInput")
    wsT_d = nc.dram_tensor("wsT", [H, H], f32, kind="ExternalInput")
    outT_d = nc.dram_tensor("outT", [H, BPC], f32, kind="ExternalInput")
    bA_d = nc.dram_tensor("bA", [128, NHT], f32, kind="ExternalInput")
    vA_d = nc.dram_tensor("vA", [128, NHT], bf16, kind="ExternalInput")
    mk_d = nc.dram_tensor("mk", [BPC, S], f32, kind="ExternalInput")
    out_d = nc.dram_tensor("out", [BPC, S], f32, kind="ExternalOutput")

    tc = _split_drain_context(nc)
    with tc:
        import contextlib

        with contextlib.ExitStack() as ctx:
            const = ctx.enter_context(tc.tile_pool(name="const", bufs=1))
            encp = ctx.enter_context(tc.tile_pool(name="encp", bufs=3))
            enrg = ctx.enter_context(tc.tile_pool(name="enrg", bufs=8))
            rowp = ctx.enter_context(tc.tile_pool(name="rowp", bufs=1))
            pe_p = ctx.enter_context(tc.tile_pool(name="pe_p", bufs=6, space="PSUM"))
            ps_p = ctx.enter_context(tc.tile_pool(name="ps_p", bufs=2, space="PSUM"))

            # ---- replicated constants -------------------------------------
            we_sb = const.tile([128, NK, H], bf16)       # W_e.T tiles [e,k,h]
            ws_sb = const.tile([128, H // 128, H], f32r)  # W_s.T tiles
            ot_sb = const.tile([128, H // 128, BPC], f32r)  # output.T tiles
            bA_sb = const.tile([128, NHT], f32)
            vA_sb = const.tile([128, NHT], bf16)
            mk_sb = const.tile([128, S], f32)            # rows 32b hold batch b

            nc.sync.dma_start(
                we_sb[:],
                weT_d.ap().rearrange("(k p) h -> p k h", p=128),
            )
            nc.sync.dma_start(
                ws_sb[:],
                wsT_d.ap().rearrange("(k p) h -> p k h", p=128).bitcast(f32r),
            )
            nc.sync.dma_start(
                ot_sb[:],
                outT_d.ap().rearrange("(k p) b -> p k b", p=128).bitcast(f32r),
            )
            nc.sync.dma_start(bA_sb[:], bA_d.ap()[:])
            nc.sync.dma_start(vA_sb[:], vA_d.ap()[:])
            nc.sync.dma_start(mk_sb[0:128:32, :], mk_d.ap()[:])

            # ---- state projection: c[ht] = output @ W_s.T + b_attn --------
            c_sb = const.tile([128, NHT, BPC], f32)
            for ht in range(NHT):
                pc = ps_p.tile([128, BPC], f32, tag="score")
                for k in range(H // 128):
                    nc.tensor.matmul(
                        pc[:],
                        ws_sb[:, k, ht * 128:(ht + 1) * 128],
                        ot_sb[:, k, :],
                        start=(k == 0),
                        stop=(k == H // 128 - 1),
                    )
                nc.vector.tensor_scalar_add(
                    c_sb[:, ht, :], pc[:], bA_sb[:, ht:ht + 1]
                )

            # ---- steady state ---------------------------------------------
            def body(_iv=None):
                scores = rowp.tile([128, S], f32, tag="scores")
                # rows besides {0,32,64,96} are never written by the score
                # path; zero them so the full-partition softmax below stays
                # finite there.
                nc.gpsimd.memset(scores[:], 0.0)
                pending = None  # deferred (b, sb, energy tiles)

                def emit_scores(p):
                    b, sb, en_tiles = p
                    sc = ps_p.tile([128, SB], f32, tag="score")
                    for ht in range(NHT):
                        nc.tensor.matmul(
                            sc[32 * b:32 * b + 1, :],
                            vA_sb[:, ht:ht + 1],
                            en_tiles[ht][:],
                            start=(ht == 0),
                            stop=(ht == NHT - 1),
                            tile_position=(0, 32 * b),
                        )
                    # scores chunk + mask term -> scores row (partition 32b)
                    nc.vector.tensor_add(
                        scores[32 * b:32 * b + 1, sb * SB:(sb + 1) * SB],
                        sc[32 * b:32 * b + 1, :],
                        mk_sb[32 * b:32 * b + 1, sb * SB:(sb + 1) * SB],
                    )

                for b in range(BPC):
                    for half in range(2):
                        et = encp.tile([128, NK, SH], bf16, tag="enc")
                        nc.sync.dma_start(
                            et[:],
                            encT_d.ap()[b]
                            .rearrange("k p s -> p k s")[:, :, half * SH:(half + 1) * SH],
                        )
                        for sbh in range(NSB // 2):
                            sb = half * (NSB // 2) + sbh
                            pe = [
                                pe_p.tile([128, SB], f32, tag="pe", name=f"pe{ht}")
                                for ht in range(NHT)
                            ]
                            # k-contiguous per PSUM bank: all 8 accumulation
                            # steps for one ht before switching banks (PSUM
                            # queue cycling stalls the PE otherwise).
                            for ht in range(NHT):
                                for k in range(NK):
                                    nc.tensor.matmul(
                                        pe[ht][:],
                                        we_sb[:, k, ht * 128:(ht + 1) * 128],
                                        et[:, k, sbh * SB:(sbh + 1) * SB],
                                        start=(k == 0),
                                        stop=(k == NK - 1),
                                    )
                            en_tiles = []
                            for ht in range(NHT):
                                en = enrg.tile([128, SB], bf16, tag="en")
                                nc.scalar.activation(
                                    en[:], pe[ht][:], Tanh,
                                    bias=c_sb[:, ht, b:b + 1], scale=1.0,
                                )
                                en_tiles.append(en)
                            if pending is not None:
                                emit_scores(pending)
                            pending = (b, sb, en_tiles)
                if pending is not None:
                    emit_scores(pending)

                # ---- softmax over S on partitions {0,32,64,96} ------------
                expv = rowp.tile([128, S], f32, tag="expv")
                ssum = rowp.tile([128, 1], f32, tag="ssum")
                rec = rowp.tile([128, 1], f32, tag="rec")
                outr = rowp.tile([128, S], f32, tag="outr")
                nc.scalar.activation(
                    expv[:], scores[:], Exp, accum_out=ssum[:]
                )
                nc.vector.reciprocal(rec[:], ssum[:])
                nc.vector.tensor_scalar_mul(outr[:], expv[:], rec[:])
                nc.sync.dma_start(out_d.ap()[:], outr[0:128:32, :])

            if reps == 1:
                body()
            else:
                from concourse import mybir as _mb

                with tc.For_i(
                    0, reps, 1,
                    hint_engines=(
                        _mb.EngineType.PE,
                        _mb.EngineType.Activation,
                        _mb.EngineType.SP,
                        _mb.EngineType.DVE,
                    ),
                ):
                    body()

    enforce_wait_limit(nc)
    return nc


def _shard_inputs(output, encoder_outputs, encoder_mask, W_attn, b_attn, v):
    import ml_dtypes

    wT32 = np.ascontiguousarray(W_attn.T.astype(np.float32))        # [1536, 512]
    weT = wT32[H:].astype(ml_dtypes.bfloat16)                       # [1024, 512]
    wsT = wT32[:H]                                                  # [512, 512]
    bA = np.ascontiguousarray(
        b_attn.astype(np.float32).reshape(NHT, 128).T
    )                                                               # [128, 4]
    vA = np.ascontiguousarray(
        v.astype(np.float32).reshape(NHT, 128).T
    ).astype(ml_dtypes.bfloat16)                                    # [128, 4]

    in_maps = []
    for c in range(N_CORES):
        b0 = c * BPC
        enc = encoder_outputs[b0:b0 + BPC]                          # [4, S, 2H]
        encT = np.ascontiguousarray(
            enc.transpose(0, 2, 1).astype(np.float32)
        ).astype(ml_dtypes.bfloat16).reshape(BPC, NK, 128, S)       # [4, 8, 128, S]
        outT = np.ascontiguousarray(
            output[b0:b0 + BPC].T.astype(np.float32)
        )                                                           # [512, 4]
        mk = (-1000.0 * (encoder_mask[b0:b0 + BPC] == 0)).astype(np.float32)
        in_maps.append(
            {"encT": encT, "weT": weT, "wsT": wsT, "outT": outT,
             "bA": bA, "vA": vA, "mk": mk}
        )
    return in_maps


def kernel(output, encoder_outputs, encoder_mask, W_attn, b_attn, v):
    from concourse.bass_utils import run_bass_kernel_spmd

    output = np.asarray(output)
    encoder_outputs = np.asarray(encoder_outputs)
    encoder_mask = np.asarray(encoder_mask)
    W_attn = np.asarray(W_attn)
    b_attn = np.asarray(b_attn)
    v = np.asarray(v)

    nc = build_nc()
    in_maps = _shard_inputs(output, encoder_outputs, encoder_mask, W_attn, b_attn, v)
    res = run_bass_kernel_spmd(nc, in_maps, core_ids=list(range(N_CORES)))
    full = np.concatenate([res.results[c]["out"] for c in range(N_CORES)], axis=0)
    return full.reshape(B, 1, S).astype(np.float32)


# revision 15
# speedup vs baseline: 1.0530x; 1.0530x over previous
"""Bass/Trainium2 kernel for nn_Attn_1185410973711 (additive attention scores).

Computation (reference, fp32):
    W_s = W_attn[:, :H]; W_e = W_attn[:, H:]
    energy  = tanh(output @ W_s.T [:,None,:] + einsum('bse,he->bsh', enc, W_e) + b_attn)
    scores  = einsum('bsh,h->bs', energy, v) - 1000*(mask==0)
    out     = softmax(scores, axis=-1)           # [B, 1, S]

Strategy: data-parallel over batch B=32 across 8 NeuronCores (4 batches per
core); W_attn/b_attn/v replicated. Host-side marshalling pre-transposes
encoder_outputs to [e, s] layout (pure layout change) so the kernel streams
contraction-ready tiles straight from DRAM with zero on-chip transposes.
The dominant matmul (enc_proj, 8.6 GFLOP/core) runs in bf16 (enc and W_e
rounded on the host) with fp32 PSUM accumulation, k-contiguous per PSUM
bank; the small state projection stays float32r. tanh(+state/bias) is
fused on the scalar engine with per-partition bias; the v-dot is a second
M=1 bf16 matmul; softmax runs on partitions {0,32,64,96} at the tail.
"""

import numpy as np

B, S, H = 32, 2048, 512
E2 = 2 * H            # 1024, encoder feature dim
N_CORES = 8
BPC = B // N_CORES    # 4 batches per core
NK = E2 // 128        # 8 contraction tiles
NHT = H // 128        # 4 hidden tiles
SB = 512              # s-block (moving free dim)
NSB = S // SB         # 4 s-blocks
SH = S // 2           # half-batch s columns per enc DMA


def _split_drain_context(nc):
    """TileContext subclass working around two walrus limits in this build:
    the kernel-tail drain and several opcodes (notably the self-loading
    fp32 matmul) reject instructions carrying more than one semaphore
    wait. See enforce_wait_limit()."""
    import concourse.tile as tile
    from concourse.vector_clock import ScopedClock

    class TileContextSplitDrain(tile.TileContext):
        def _drain_and_barrier(self, tick_clock, wait_clock):
            probe = self.nc.sync.nop(nofuse=True, hint="tail_wait_probe")
            wait_clock.add_sem_waits(
                probe.ins, ScopedClock({None: tick_clock.global_clock})
            )
            si = probe.ins.sync_info
            waits = list(si.on_wait or []) if si is not None else []
            if si is not None:
                si.on_wait.clear()
            by_name = {h.name: h for h in self.sems.allocated().values()}
            for w in waits:
                h = by_name.get(w.ant_name)
                assert h is not None, f"missing semaphore handle for {w.ant_name}"
                self.nc.sync.wait_ge(h, w.wait_value)
            self.nc.sync.drain()
            self.nc.all_engine_barrier()
            popped = self.nc._tile_sem_poison_stack.pop()
            assert popped is self._sem_poison
            self.nc.clear_and_free_semaphores(list(self.sems.allocated().values()))
            self.nc.all_engine_barrier()

    return TileContextSplitDrain(nc)


def enforce_wait_limit(nc, limit=1):
    """Hoist excess semaphore waits onto inserted same-engine event-sem wait
    instructions placed immediately before the over-budget instruction.
    In-order engine execution makes an earlier wait strictly conservative,
    so this is always sound."""
    import copy

    template = None
    for fn in nc.m.functions:
        for bb in fn.blocks:
            for ins in bb.instructions:
                if type(ins).__name__ == "InstEventSemaphore":
                    si = ins.sync_info
                    if si and si.on_wait and len(si.on_wait) == 1:
                        template = ins
                        break
            if template:
                break
        if template:
            break

    n_new = 0
    for fn in nc.m.functions:
        for bb in fn.blocks:
            il = bb.instructions
            new_il = []
            changed = False
            for ins in il:
                si = ins.sync_info
                waits = list(si.on_wait) if si and si.on_wait else []
                if len(waits) > limit and type(ins).__name__ != "InstEventSemaphore":
                    assert template is not None, "no event-sem template found"
                    for w in waits[limit:]:
                        c = copy.deepcopy(template)
                        n_new += 1
                        c.name = f"I-waitfix-{n_new}"
                        c.engine = ins.engine
                        csi = c.sync_info
                        csi.on_wait.clear()
                        csi.on_wait.append(w)
                        csi.on_update.clear()
                        new_il.append(c)
                    si.on_wait.clear()
                    for w in waits[:limit]:
                        si.on_wait.append(w)
                    changed = True
                new_il.append(ins)
            if changed:
                il[:] = new_il
    return n_new


def build_nc(reps=1):
    """Build the per-core Bass program. reps>1 wraps the steady-state body in
    a For_i loop re-running the identical computation (for timing)."""
    import concourse.bass as bass
    from concourse import mybir

    f32 = mybir.dt.float32
    f32r = mybir.dt.float32r
    bf16 = mybir.dt.bfloat16
    Tanh = mybir.ActivationFunctionType.Tanh
    Exp = mybir.ActivationFunctionType.Exp

    nc = bass.Bass("TRN2", target_bir_lowering=False, debug=False)

    encT_d = nc.dram_tensor("encT", [BPC, NK, 128, S], bf16, kind="ExternalInput")
    weT_d = nc.dram_tensor("weT", [2 * H, H], bf16, kind="ExternalInput")
    wsT_d = nc.dram_tensor("wsT", [H, H], f32, kind="ExternalInput")
    outT_d = nc.dram_tensor("outT", [H, BPC], f32, kind="ExternalInput")
    bA_d = nc.dram_tensor("bA", [128, NHT], f32, kind="ExternalInput")
    vA_d = nc.dram_tensor("vA", [128, NHT], bf16, kind="ExternalInput")
    mk_d = nc.dram_tensor("mk", [BPC, S], f32, kind="ExternalInput")
    out_d = nc.dram_tensor("out", [BPC, S], f32, kind="ExternalOutput")

    tc = _split_drain_context(nc)
    with tc:
        import contextlib

        with contextlib.ExitStack() as ctx:
            const = ctx.enter_context(tc.tile_pool(name="const", bufs=1))
            encp = ctx.enter_context(tc.tile_pool(name="encp", bufs=5))
            enrg = ctx.enter_context(tc.tile_pool(name="enrg", bufs=12))
            rowp = ctx.enter_context(tc.tile_pool(name="rowp", bufs=1))
            pe_p = ctx.enter_context(tc.tile_pool(name="pe_p", bufs=6, space="PSUM"))
            ps_p = ctx.enter_context(tc.tile_pool(name="ps_p", bufs=2, space="PSUM"))

            # ---- replicated constants -------------------------------------
            we_sb = const.tile([128, NK, H], bf16)       # W_e.T tiles [e,k,h]
            ws_sb = const.tile([128, H // 128, H], f32r)  # W_s.T tiles
            ot_sb = const.tile([128, H // 128, BPC], f32r)  # output.T tiles
            bA_sb = const.tile([128, NHT], f32)
            vA_sb = const.tile([128, NHT], bf16)
            mk_sb = const.tile([128, S], f32)            # rows 32b hold batch b

            nc.sync.dma_start(
                we_sb[:],
                weT_d.ap().rearrange("(k p) h -> p k h", p=128),
            )
            nc.sync.dma_start(
                ws_sb[:],
                wsT_d.ap().rearrange("(k p) h -> p k h", p=128).bitcast(f32r),
            )
            nc.sync.dma_start(
                ot_sb[:],
                outT_d.ap().rearrange("(k p) b -> p k b", p=128).bitcast(f32r),
            )
            nc.sync.dma_start(bA_sb[:], bA_d.ap()[:])
            nc.sync.dma_start(vA_sb[:], vA_d.ap()[:])
            nc.sync.dma_start(mk_sb[0:128:32, :], mk_d.ap()[:])

            # ---- state projection: c[ht] = output @ W_s.T + b_attn --------
            # The k-loop is repeated with start=True on each pass: only the
            # last pass's accumulation survives, so the result is unchanged
            # while the extra matmuls keep the PE busy during the initial enc
            # DMA and push the HAM activity window to full clock before the
            # main matmul stream begins.
            WARM_REPS = 6
            c_sb = const.tile([128, NHT, BPC], f32)
            for ht in range(NHT):
                pc = ps_p.tile([128, BPC], f32, tag="score")
                for _rep in range(WARM_REPS):
                    for k in range(H // 128):
                        nc.tensor.matmul(
                            pc[:],
                            ws_sb[:, k, ht * 128:(ht + 1) * 128],
                            ot_sb[:, k, :],
                            start=(k == 0),
                            stop=(k == H // 128 - 1),
                        )
                nc.vector.tensor_scalar_add(
                    c_sb[:, ht, :], pc[:], bA_sb[:, ht:ht + 1]
                )

            # ---- steady state ---------------------------------------------
            def body(_iv=None):
                scores = rowp.tile([128, S], f32, tag="scores")
                # rows besides {0,32,64,96} are never written by the score
                # path; zero them so the full-partition softmax below stays
                # finite there.
                nc.gpsimd.memset(scores[:], 0.0)
                pending = None  # deferred (b, sb, energy tiles)

                def emit_scores(p):
                    b, sb, en_tiles = p
                    sc = ps_p.tile([128, SB], f32, tag="score")
                    for ht in range(NHT):
                        nc.tensor.matmul(
                            sc[32 * b:32 * b + 1, :],
                            vA_sb[:, ht:ht + 1],
                            en_tiles[ht][:],
                            start=(ht == 0),
                            stop=(ht == NHT - 1),
                            tile_position=(0, 32 * b),
                        )
                    # scores chunk + mask term -> scores row (partition 32b)
                    nc.vector.tensor_add(
                        scores[32 * b:32 * b + 1, sb * SB:(sb + 1) * SB],
                        sc[32 * b:32 * b + 1, :],
                        mk_sb[32 * b:32 * b + 1, sb * SB:(sb + 1) * SB],
                    )

                for b in range(BPC):
                    for half in range(2):
                        et = encp.tile([128, NK, SH], bf16, tag="enc")
                        src = (
                            encT_d.ap()[b]
                            .rearrange("k p s -> p k s")[:, :, half * SH:(half + 1) * SH]
                        )
                        if b == 0 and half == 0:
                            # split the very first load per k-tile so the k=0
                            # matmuls start after ~1/8 of the transfer instead
                            # of waiting for the whole 2 MB
                            for k in range(NK):
                                nc.sync.dma_start(et[:, k, :], src[:, k, :])
                        else:
                            nc.sync.dma_start(et[:], src)
                        for sbh in range(NSB // 2):
                            sb = half * (NSB // 2) + sbh
                            pe = [
                                pe_p.tile([128, SB], f32, tag="pe", name=f"pe{ht}")
                                for ht in range(NHT)
                            ]
                            # k-contiguous per PSUM bank: all 8 accumulation
                            # steps for one ht before switching banks (PSUM
                            # queue cycling stalls the PE otherwise).
                            for ht in range(NHT):
                                for k in range(NK):
                                    nc.tensor.matmul(
                                        pe[ht][:],
                                        we_sb[:, k, ht * 128:(ht + 1) * 128],
                                        et[:, k, sbh * SB:(sbh + 1) * SB],
                                        start=(k == 0),
                                        stop=(k == NK - 1),
                                    )
                            en_tiles = []
                            for ht in range(NHT):
                                en = enrg.tile([128, SB], bf16, tag="en")
                                nc.scalar.activation(
                                    en[:], pe[ht][:], Tanh,
                                    bias=c_sb[:, ht, b:b + 1], scale=1.0,
                                )
                                en_tiles.append(en)
                            if pending is not None:
                                emit_scores(pending)
                            pending = (b, sb, en_tiles)
                if pending is not None:
                    emit_scores(pending)

                # ---- softmax over S on partitions {0,32,64,96} ------------
                expv = rowp.tile([128, S], f32, tag="expv")
                ssum = rowp.tile([128, 1], f32, tag="ssum")
                rec = rowp.tile([128, 1], f32, tag="rec")
                outr = rowp.tile([128, S], f32, tag="outr")
                nc.scalar.activation(
                    expv[:], scores[:], Exp, accum_out=ssum[:]
                )
                nc.vector.reciprocal(rec[:], ssum[:])
                nc.vector.tensor_scalar_mul(outr[:], expv[:], rec[:])
                nc.sync.dma_start(out_d.ap()[:], outr[0:128:32, :])

            if reps == 1:
                body()
            else:
                from concourse import mybir as _mb

                with tc.For_i(
                    0, reps, 1,
                    hint_engines=(
                        _mb.EngineType.PE,
                        _mb.EngineType.Activation,
                        _mb.EngineType.SP,
                        _mb.EngineType.DVE,
                    ),
                ):
                    body()

    enforce_wait_limit(nc)
    return nc


def _shard_inputs(output, encoder_outputs, encoder_mask, W_attn, b_attn, v):
    import ml_dtypes

    wT32 = np.ascontiguousarray(W_attn.T.astype(np.float32))        # [1536, 512]
    weT = wT32[H:].astype(ml_dtypes.bfloat16)                       # [1024, 512]
    wsT = wT32[:H]                                                  # [512, 512]
    bA = np.ascontiguousarray(
        b_attn.astype(np.float32).reshape(NHT, 128).T
    )                                                               # [128, 4]
    vA = np.ascontiguousarray(
        v.astype(np.float32).reshape(NHT, 128).T
    ).astype(ml_dtypes.bfloat16)                                    # [128, 4]

    in_maps = []
    for c in range(N_CORES):
        b0 = c * BPC
        enc = encoder_outputs[b0:b0 + BPC]                          # [4, S, 2H]
        encT = np.ascontiguousarray(
            enc.transpose(0, 2, 1).astype(np.float32)
        ).astype(ml_dtypes.bfloat16).reshape(BPC, NK, 128, S)       # [4, 8, 128, S]
        outT = np.ascontiguousarray(
            output[b0:b0 + BPC].T.astype(np.float32)
        )                                                           # [512, 4]
        mk = (-1000.0 * (encoder_mask[b0:b0 + BPC] == 0)).astype(np.float32)
        in_maps.append(
            {"encT": encT, "weT": weT, "wsT": wsT, "outT": outT,
             "bA": bA, "vA": vA, "mk": mk}
        )
    return in_maps


def kernel(output, encoder_outputs, encoder_mask, W_attn, b_attn, v):
    from concourse.bass_utils import run_bass_kernel_spmd

    output = np.asarray(output)
    encoder_outputs = np.asarray(encoder_outputs)
    encoder_mask = np.asarray(encoder_mask)
    W_attn = np.asarray(W_attn)
    b_attn = np.asarray(b_attn)
    v = np.asarray(v)

    nc = build_nc()
    in_maps = _shard_inputs(output, encoder_outputs, encoder_mask, W_attn, b_attn, v)
    res = run_bass_kernel_spmd(nc, in_maps, core_ids=list(range(N_CORES)))
    full = np.concatenate([res.results[c]["out"] for c in range(N_CORES)], axis=0)
    return full.reshape(B, 1, S).astype(np.float32)


# revision 18
# speedup vs baseline: 1.1239x; 1.0673x over previous
"""Bass/Trainium2 kernel for nn_Attn_1185410973711 (additive attention scores).

Computation (reference, fp32):
    W_s = W_attn[:, :H]; W_e = W_attn[:, H:]
    energy  = tanh(output @ W_s.T [:,None,:] + einsum('bse,he->bsh', enc, W_e) + b_attn)
    scores  = einsum('bsh,h->bs', energy, v) - 1000*(mask==0)
    out     = softmax(scores, axis=-1)           # [B, 1, S]

Strategy: data-parallel over batch B=32 across 8 NeuronCores (4 batches per
core); W_attn/b_attn/v replicated. Host-side marshalling pre-transposes
encoder_outputs to [e, s] layout (pure layout change; any matmul contracting
the feature axis needs it on partitions for both operands) so the kernel
streams contraction-ready tiles straight from DRAM with zero on-chip
transposes. The dominant matmul (enc_proj, 8.6 GFLOP/core) runs in bf16
(enc and W_e rounded on the host) with fp32 PSUM accumulation, oriented
with enc tiles stationary and W_e moving so PSUM lands as [s_part, h_free]:
the v-dot then leaves the PE entirely (DVE multiply + scalar-engine
accumulate along the free axis) instead of costing 64 extra M=1 matmul
streams. The state projection runs as fp32r matmuls on host-replicated
operands, doubling as the PE HAM warm-up burst. Per-batch softmax uses one
PE transpose of the [128,16] score columns, exp with fused row-sums, and
two tiny ones-matmuls for the cross-partition total and broadcast.
"""

import contextlib

import numpy as np

B, S, H = 32, 2048, 512
E2 = 2 * H            # 1024, encoder feature dim
N_CORES = 8
BPC = B // N_CORES    # 4 batches per core
NK = E2 // 128        # 8 contraction tiles
SH = S // 2           # s columns per enc DMA (half batch)
NST = S // 128        # 16 s-tiles per batch
NSTH = SH // 128      # 8 s-tiles per half


def _split_drain_context(nc):
    """TileContext subclass working around a walrus limit in this build: the
    kernel-tail drain rejects instructions carrying more than one semaphore
    wait. See enforce_wait_limit()."""
    import concourse.tile as tile
    from concourse.vector_clock import ScopedClock

    class TileContextSplitDrain(tile.TileContext):
        def _drain_and_barrier(self, tick_clock, wait_clock):
            probe = self.nc.sync.nop(nofuse=True, hint="tail_wait_probe")
            wait_clock.add_sem_waits(
                probe.ins, ScopedClock({None: tick_clock.global_clock})
            )
            si = probe.ins.sync_info
            waits = list(si.on_wait or []) if si is not None else []
            if si is not None:
                si.on_wait.clear()
            by_name = {h.name: h for h in self.sems.allocated().values()}
            for w in waits:
                h = by_name.get(w.ant_name)
                assert h is not None, f"missing semaphore handle for {w.ant_name}"
                self.nc.sync.wait_ge(h, w.wait_value)
            self.nc.sync.drain()
            self.nc.all_engine_barrier()
            popped = self.nc._tile_sem_poison_stack.pop()
            assert popped is self._sem_poison
            self.nc.clear_and_free_semaphores(list(self.sems.allocated().values()))
            self.nc.all_engine_barrier()

    return TileContextSplitDrain(nc)


def enforce_wait_limit(nc, limit=1):
    """Hoist excess semaphore waits onto inserted same-engine event-sem wait
    instructions placed immediately before the over-budget instruction.
    In-order engine execution makes an earlier wait strictly conservative,
    so this is always sound. Several opcodes in this walrus build (notably
    self-loading fp32 matmuls and Drain) reject multi-wait encodings."""
    import copy

    template = None
    for fn in nc.m.functions:
        for bb in fn.blocks:
            for ins in bb.instructions:
                if type(ins).__name__ == "InstEventSemaphore":
                    si = ins.sync_info
                    if si and si.on_wait and len(si.on_wait) == 1:
                        template = ins
                        break
            if template:
                break
        if template:
            break

    n_new = 0
    for fn in nc.m.functions:
        for bb in fn.blocks:
            il = bb.instructions
            new_il = []
            changed = False
            for ins in il:
                si = ins.sync_info
                waits = list(si.on_wait) if si and si.on_wait else []
                if len(waits) > limit and type(ins).__name__ != "InstEventSemaphore":
                    assert template is not None, "no event-sem template found"
                    for w in waits[limit:]:
                        c = copy.deepcopy(template)
                        n_new += 1
                        c.name = f"I-waitfix-{n_new}"
                        c.engine = ins.engine
                        csi = c.sync_info
                        csi.on_wait.clear()
                        csi.on_wait.append(w)
                        csi.on_update.clear()
                        new_il.append(c)
                    si.on_wait.clear()
                    for w in waits[:limit]:
                        si.on_wait.append(w)
                    changed = True
                new_il.append(ins)
            if changed:
                il[:] = new_il
    return n_new


def build_nc(reps=1):
    """Build the per-core Bass program. reps>1 wraps the steady-state body in
    a For_i loop re-running the identical computation (for timing)."""
    import concourse.bass as bass
    from concourse import mybir

    f32 = mybir.dt.float32
    f32r = mybir.dt.float32r
    bf16 = mybir.dt.bfloat16
    Tanh = mybir.ActivationFunctionType.Tanh
    Exp = mybir.ActivationFunctionType.Exp
    Ident = mybir.ActivationFunctionType.Identity

    nc = bass.Bass("TRN2", target_bir_lowering=False, debug=False)

    encT_d = nc.dram_tensor("encT", [BPC, NK, 128, S], bf16, kind="ExternalInput")
    weT_d = nc.dram_tensor("weT", [2 * H, H], bf16, kind="ExternalInput")
    wsT_d = nc.dram_tensor("wsT", [H, H], bf16, kind="ExternalInput")
    outB_d = nc.dram_tensor("outB", [BPC, 4, 128, 128], bf16, kind="ExternalInput")
    bAR_d = nc.dram_tensor("bAR", [128, H], f32, kind="ExternalInput")
    vR_d = nc.dram_tensor("vR", [128, H], bf16, kind="ExternalInput")
    mk2_d = nc.dram_tensor("mk2", [BPC, NST, 128], f32, kind="ExternalInput")
    eye_d = nc.dram_tensor("eye", [128, 128], f32, kind="ExternalInput")
    out_d = nc.dram_tensor("out", [BPC, S], f32, kind="ExternalOutput")

    tc = _split_drain_context(nc)
    with tc:
        with contextlib.ExitStack() as ctx:
            const = ctx.enter_context(tc.tile_pool(name="const", bufs=1))
            encp = ctx.enter_context(tc.tile_pool(name="encp", bufs=6))
            prep = ctx.enter_context(tc.tile_pool(name="prep", bufs=6))
            enrg = ctx.enter_context(tc.tile_pool(name="enrg", bufs=6))
            scrp = ctx.enter_context(tc.tile_pool(name="scrp", bufs=3))
            rowp = ctx.enter_context(tc.tile_pool(name="rowp", bufs=1))
            pe_p = ctx.enter_context(tc.tile_pool(name="pe_p", bufs=7, space="PSUM"))
            ms_p = ctx.enter_context(tc.tile_pool(name="ms_p", bufs=1, space="PSUM"))

            we_sb = const.tile([128, NK, H], bf16)        # W_e.T tiles [e,k,h]
            ws_sb = const.tile([128, H // 128, H], bf16)  # W_s.T tiles
            ob_sb = const.tile([128, BPC, H // 128, 128], bf16)  # output bcast
            bAR_sb = const.tile([128, H], f32)
            vR_sb = const.tile([128, H], bf16)
            mk_sb = const.tile([128, BPC, NST], f32)
            eye_sb = const.tile([128, 128], f32)
            ones16 = const.tile([16, 1], f32)
            ones1 = const.tile([1, 16], f32)

            nc.sync.dma_start(we_sb[:], weT_d.ap().rearrange("(k p) h -> p k h", p=128))
            nc.sync.dma_start(
                ws_sb[:], wsT_d.ap().rearrange("(k p) h -> p k h", p=128)
            )
            nc.sync.dma_start(
                ob_sb[:], outB_d.ap().rearrange("b k p m -> p b k m")
            )
            nc.sync.dma_start(bAR_sb[:], bAR_d.ap()[:])
            nc.sync.dma_start(vR_sb[:], vR_d.ap()[:])
            nc.sync.dma_start(mk_sb[:], mk2_d.ap().rearrange("b t p -> p b t"))
            nc.sync.dma_start(eye_sb[:], eye_d.ap()[:])
            nc.gpsimd.memset(ones16[:], 1.0)
            nc.gpsimd.memset(ones1[:], 1.0)

            # ---- c_rep[b] = broadcast(output[b] @ W_s.T + b_attn) ---------
            # outB is output[b] replicated along M on the host, so the state
            # matmul directly yields the row-broadcast [128, H] result; also
            # serves as the PE warm-up burst during the first enc DMA.
            c_rep = const.tile([128, BPC, H], f32)
            for b in range(BPC):
                pc = ms_p.tile([128, H], f32, tag="misc", name=f"pc{b}")
                for k in range(H // 128):
                    nc.tensor.matmul(
                        pc[:],
                        ob_sb[:, b, k, :],
                        ws_sb[:, k, :],
                        start=(k == 0),
                        stop=(k == H // 128 - 1),
                    )
                nc.vector.tensor_add(c_rep[:, b, :], pc[:], bAR_sb[:])

            def body(_iv=None):
                sccols = rowp.tile([128, BPC, NST], f32, tag="sccols")
                expv = rowp.tile([16, BPC * 128], f32, tag="expv")
                accT = rowp.tile([16, BPC], f32, tag="accT")
                outv = rowp.tile([16, BPC * 128], f32, tag="outv")

                for b in range(BPC):
                    for half in range(2):
                        et = encp.tile([128, NK, SH], bf16, tag="enc")
                        src = (
                            encT_d.ap()[b]
                            .rearrange("k p s -> p k s")[:, :, half * SH:(half + 1) * SH]
                        )
                        if b == 0 and half == 0:
                            # split the very first load per k-tile so the k=0
                            # matmuls start after ~1/8 of the transfer
                            for k in range(NK):
                                nc.sync.dma_start(et[:, k, :], src[:, k, :])
                        else:
                            nc.sync.dma_start(et[:], src)
                        for sth in range(NSTH):
                            st = half * NSTH + sth
                            ps = pe_p.tile([128, H], f32, tag="pe")
                            for k in range(NK):
                                nc.tensor.matmul(
                                    ps[:],
                                    et[:, k, sth * 128:(sth + 1) * 128],
                                    we_sb[:, k, :],
                                    start=(k == 0),
                                    stop=(k == NK - 1),
                                )
                            pre = prep.tile([128, H], f32, tag="pre")
                            nc.vector.tensor_add(pre[:], ps[:], c_rep[:, b, :])
                            en = enrg.tile([128, H], bf16, tag="en")
                            nc.scalar.activation(en[:], pre[:], Tanh)
                            scr = scrp.tile([128, H], bf16, tag="scr")
                            nc.vector.tensor_mul(scr[:], en[:], vR_sb[:])
                            dmp = scrp.tile([128, H], bf16, tag="dmp")
                            nc.scalar.activation(
                                dmp[:], scr[:], Ident,
                                accum_out=sccols[:, b, st:st + 1],
                            )
                    # ---- per-batch softmax numerator ----------------------
                    nc.vector.tensor_add(
                        sccols[:, b, :], sccols[:, b, :], mk_sb[:, b, :]
                    )
                    tp = ms_p.tile([16, 128], f32, tag="misc", name=f"tp{b}")
                    nc.tensor.transpose(tp[:], sccols[:, b, :], eye_sb[:])
                    nc.scalar.activation(
                        expv[:, b * 128:(b + 1) * 128], tp[:], Exp,
                        accum_out=accT[:, b:b + 1],
                    )

                # ---- normalize: per-b total over the 16 partitions --------
                tot = ms_p.tile([1, BPC], f32, tag="misc", name="tot")
                nc.tensor.matmul(tot[:], ones16[:], accT[:], start=True, stop=True)
                rec4 = rowp.tile([1, BPC], f32, tag="rec4")
                nc.vector.reciprocal(rec4[:], tot[:])
                rb = ms_p.tile([16, BPC], f32, tag="misc", name="rb")
                nc.tensor.matmul(rb[:], ones1[:], rec4[:], start=True, stop=True)
                rec_sb = rowp.tile([16, BPC], f32, tag="rec_sb")
                nc.vector.tensor_copy(rec_sb[:], rb[:])
                for b in range(BPC):
                    nc.vector.tensor_scalar_mul(
                        outv[:, b * 128:(b + 1) * 128],
                        expv[:, b * 128:(b + 1) * 128],
                        rec_sb[:, b:b + 1],
                    )
                    nc.sync.dma_start(
                        out_d.ap()[b].rearrange("(t s) -> t s", t=16),
                        outv[:, b * 128:(b + 1) * 128],
                    )

            if reps == 1:
                body()
            else:
                from concourse import mybir as _mb

                with tc.For_i(
                    0, reps, 1,
                    hint_engines=(
                        _mb.EngineType.PE, _mb.EngineType.Activation,
                        _mb.EngineType.SP, _mb.EngineType.DVE,
                    ),
                ):
                    body()

    enforce_wait_limit(nc)
    return nc


def _shard_inputs(output, encoder_outputs, encoder_mask, W_attn, b_attn, v):
    import ml_dtypes

    wT32 = np.ascontiguousarray(W_attn.T.astype(np.float32))        # [1536, 512]
    weT = wT32[H:].astype(ml_dtypes.bfloat16)                       # [1024, 512]
    wsT = wT32[:H].astype(ml_dtypes.bfloat16)                       # [512, 512]
    eye = np.eye(128, dtype=np.float32)
    bAR = np.broadcast_to(b_attn.astype(np.float32), (128, H)).copy()
    vR = np.broadcast_to(
        v.astype(np.float32).astype(ml_dtypes.bfloat16), (128, H)
    ).copy()

    in_maps = []
    for c in range(N_CORES):
        b0 = c * BPC
        enc = encoder_outputs[b0:b0 + BPC]                          # [4, S, 2H]
        encT = np.ascontiguousarray(
            enc.transpose(0, 2, 1).astype(np.float32)
        ).astype(ml_dtypes.bfloat16).reshape(BPC, NK, 128, S)       # [4, 8, 128, S]
        outB = np.broadcast_to(
            output[b0:b0 + BPC].astype(np.float32).astype(
                ml_dtypes.bfloat16
            ).reshape(BPC, 4, 128, 1),
            (BPC, 4, 128, 128),
        ).copy()
        mk2 = (-1000.0 * (encoder_mask[b0:b0 + BPC] == 0)).astype(
            np.float32
        ).reshape(BPC, NST, 128)
        in_maps.append({
            "encT": encT, "weT": weT, "wsT": wsT, "outB": outB,
            "bAR": bAR, "vR": vR, "mk2": mk2, "eye": eye,
        })
    return in_maps


def kernel(output, encoder_outputs, encoder_mask, W_attn, b_attn, v):
    from concourse.bass_utils import run_bass_kernel_spmd

    output = np.asarray(output)
    encoder_outputs = np.asarray(encoder_outputs)
    encoder_mask = np.asarray(encoder_mask)
    W_attn = np.asarray(W_attn)
    b_attn = np.asarray(b_attn)
    v = np.asarray(v)

    nc = build_nc()
    in_maps = _shard_inputs(output, encoder_outputs, encoder_mask, W_attn, b_attn, v)
    res = run_bass_kernel_spmd(nc, in_maps, core_ids=list(range(N_CORES)))
    full = np.concatenate([res.results[c]["out"] for c in range(N_CORES)], axis=0)
    return full.reshape(B, 1, S).astype(np.float32)
